# revision 17
# baseline (speedup 1.0000x reference)
"""BatchSiren Trainium2 kernel, v2.

B=2048 independent SIREN MLPs (2->32->32->3, sin activations, w0=30),
each evaluated on the same N=1024 coordinate grid. Pure data parallel
over 8 cores, 256 nets/core; 16 supergroups (sg) of 16 nets; two
point-halves (h) of 512.

Core idea: range reduction for sin is done INSIDE the PE array via a
"magic sandwich" of chained accumulating matmuls into the same PSUM
bank:  [+z] [+MAGIC] [-MAGIC] [-z]  leaves  round(z) - z = -r  in PSUM
(each matmul's contribution is one fp32 add into PSUM; the +MAGIC add
rounds the accumulator to an integer, bf16 holds MAGIC exactly).
ScalarE then does a single Sin pass per element (the only per-element
engine pass in the whole kernel): sin(-2pi * (-r)) = sin(2pi z).

L1 uses a single K=8 bf16 matmul with hi/lo-split weights AND coords
(w = wh + wl, x = xh + xl, dropping the lo*lo term) which reproduces
the fp32 product to ~2^-18 relative - needed because L1 errors amplify
~17x through layer 2. L2/L3 run in fp16 (errors don't amplify).

Per (sg,h) iteration:
  L1: 4-way row-group packs: [z K=8 bf16][+M K=1][-M K=1][-z K=8]
      -> PZ1 [128,2048] (4 banks); ACT Sin FD=2048 -> H1 f16
  L2: per bank b: [+b2 K=1 f16][z K=32 f16 16-tile][+M][-M][-z][-b2]
      -> PZ2 [128,2048]; ACT Sin FD=2048 -> H2 f16
  L3: 4 col-tiled K=128 f16 matmuls -> PC [128,512]; DVE ts adds b3
      (per-partition scalar) and copies to SBUF; DMA out.
"""
import numpy as np
import ml_dtypes

import concourse.bacc as bacc
import concourse.bass as bass
import concourse.mybir as mybir
import concourse.tile as tile
from concourse import bass_utils
from bass_rust import add_dep_helper

f32 = mybir.dt.float32
f16 = mybir.dt.float16
bf16 = mybir.dt.bfloat16
AF = mybir.ActivationFunctionType
ALU = mybir.AluOpType

W0 = 30.0
MAGIC = float(1.5 * 2 ** 23)
TWO_PI = float(2.0 * np.pi)
N_CORES = 8
B, N, IN, H, OUT = 2048, 1024, 2, 32, 3
BPC = B // N_CORES        # 256 nets per core
SGS = BPC // 16           # 16 supergroups of 16 nets
NH = N // 2               # 512 points per half

_compiled = None


def _build_module():
    nc = bacc.Bacc("TRN2", target_bir_lowering=False, debug=False)

    d_w1 = nc.dram_tensor("w1aug", [4, 9, 128 * SGS], bf16, kind="ExternalInput")
    d_w2 = nc.dram_tensor("w2s", [4, 32, 128 * SGS], f16, kind="ExternalInput")
    d_b2 = nc.dram_tensor("b2p", [4, 128 * SGS], f16, kind="ExternalInput")
    d_w3 = nc.dram_tensor("w3blk", [4, 32, 48 * SGS], f16, kind="ExternalInput")
    d_b3 = nc.dram_tensor("b3c", [128, SGS], f32, kind="ExternalInput")
    d_c = nc.dram_tensor("coords", [4, 9, N], bf16, kind="ExternalInput")
    d_out = nc.dram_tensor("out", [SGS, 2, 48, NH], f32, kind="ExternalOutput")

    with tile.TileContext(nc, pool_alloc_mode="queue") as tc:
        with tc.tile_pool(name="const", bufs=1) as cp, \
             tc.tile_pool(name="acts", bufs=3) as ap, \
             tc.tile_pool(name="outp", bufs=8) as op_, \
             tc.tile_pool(name="ps", bufs=1, space="PSUM") as ps:

            # ---- persistent constants ----
            # L1-critical pieces first so iteration 0 starts ASAP:
            c4x = cp.tile([128, N], bf16, tag="c4x")
            nc.vector.memset(c4x[:], 0.0)
            for a in range(4):
                nc.sync.dma_start(c4x[32 * a:32 * a + 9, :], d_c[a])
            mgP = cp.tile([128, 128], bf16, tag="mgP")
            nc.vector.memset(mgP[:], MAGIC)
            mgN = cp.tile([128, 128], bf16, tag="mgN")
            nc.vector.memset(mgN[:], -MAGIC)
            onesB = cp.tile([128, NH], bf16, tag="onesB")
            nc.vector.memset(onesB[:], 1.0)
            onesH = cp.tile([128, NH], f16, tag="onesH")
            nc.vector.memset(onesH[:], 1.0)

            w1p = cp.tile([128, 128 * SGS], bf16, tag="w1p")
            for a in range(4):
                nc.sync.dma_start(w1p[32 * a:32 * a + 9, :], d_w1[a])
            w1n = cp.tile([128, 128 * SGS], bf16, tag="w1n")
            nc.vector.tensor_scalar(w1n[:], w1p[:], -1.0, None, ALU.mult)

            b2p = cp.tile([128, 128 * SGS], f16, tag="b2p")
            for b in range(4):
                nc.sync.dma_start(b2p[32 * b:32 * b + 1, :], d_b2[b])
            w2p = cp.tile([128, 128 * SGS], f16, tag="w2p")
            for b in range(4):
                nc.sync.dma_start(w2p[32 * b:32 * b + 32, :], d_w2[b])
            b2n = cp.tile([128, 128 * SGS], f16, tag="b2n")
            nc.vector.tensor_scalar(b2n[:], b2p[:], -1.0, None, ALU.mult)
            w2n = cp.tile([128, 128 * SGS], f16, tag="w2n")
            nc.vector.tensor_scalar(w2n[:], w2p[:], -1.0, None, ALU.mult)

            w3sb = cp.tile([128, 48 * SGS], f16, tag="w3")
            for a in range(4):
                nc.sync.dma_start(w3sb[32 * a:32 * a + 32, :], d_w3[a])
            b3sb = cp.tile([128, SGS], f32, tag="b3")
            nc.sync.dma_start(b3sb[:], d_b3[:])

            def l1_chain(sg, h, PZ1):
                last = None
                # per bank a: [z;+M K=9] [-M K=1] [-z K=8]; the +M row
                # rounds INSIDE the K=9 matmul (internal row-order accum).
                for a in range(4):
                    sl = slice(512 * a, 512 * a + 512)
                    nc.tensor.matmul(
                        out=PZ1[:, sl],
                        lhsT=w1p[32 * a:32 * a + 9,
                                 128 * sg:128 * sg + 128],
                        rhs=c4x[32 * a:32 * a + 9, NH * h:NH * h + NH],
                        start=True, stop=False, tile_position=(32 * a, 0))
                for a in range(4):
                    sl = slice(512 * a, 512 * a + 512)
                    nc.tensor.matmul(
                        out=PZ1[:, sl], lhsT=mgN[32 * a:32 * a + 1, :],
                        rhs=onesB[32 * a:32 * a + 1, :],
                        start=False, stop=False, tile_position=(32 * a, 0))
                for a in range(4):
                    sl = slice(512 * a, 512 * a + 512)
                    last = nc.tensor.matmul(
                        out=PZ1[:, sl],
                        lhsT=w1n[32 * a:32 * a + 8,
                                 128 * sg:128 * sg + 128],
                        rhs=c4x[32 * a:32 * a + 8, NH * h:NH * h + NH],
                        start=False, stop=True, tile_position=(32 * a, 0))
                return last

            def l2_chain(sg, h, H1, PZ2, after=None):
                first = None
                for b in range(4):
                    sl = slice(512 * b, 512 * b + 512)
                    mm = nc.tensor.matmul(
                        out=PZ2[:, sl], lhsT=b2p[32 * b:32 * b + 1,
                                                 128 * sg:128 * sg + 128],
                        rhs=onesH[32 * b:32 * b + 1, :],
                        start=True, stop=False, tile_position=(32 * b, 0))
                    if first is None:
                        first = mm
                # z: 16 tiles (b strip, a colgroup); a-outer so tiles
                # reading H1's first half can start before the second
                # Sin-half completes (subtile deps).
                for a in range(4):
                    for b in range(4):
                        nc.tensor.matmul(
                            out=PZ2[32 * a:32 * a + 32, 512 * b:512 * b + 512],
                            lhsT=w2p[32 * b:32 * b + 32,
                                     128 * sg + 32 * a:128 * sg + 32 * a + 32],
                            rhs=H1[32 * b:32 * b + 32, 512 * a:512 * a + 512],
                            start=False, stop=False,
                            tile_position=(32 * b, 32 * a))
                for b in range(4):
                    sl = slice(512 * b, 512 * b + 512)
                    nc.tensor.matmul(
                        out=PZ2[:, sl], lhsT=mgP[32 * b:32 * b + 1, :],
                        rhs=onesB[32 * b:32 * b + 1, :],
                        start=False, stop=False, tile_position=(32 * b, 0))
                for b in range(4):
                    sl = slice(512 * b, 512 * b + 512)
                    nc.tensor.matmul(
                        out=PZ2[:, sl], lhsT=mgN[32 * b:32 * b + 1, :],
                        rhs=onesB[32 * b:32 * b + 1, :],
                        start=False, stop=False, tile_position=(32 * b, 0))
                for b in range(4):
                    for a in range(4):
                        nc.tensor.matmul(
                            out=PZ2[32 * a:32 * a + 32, 512 * b:512 * b + 512],
                            lhsT=w2n[32 * b:32 * b + 32,
                                     128 * sg + 32 * a:128 * sg + 32 * a + 32],
                            rhs=H1[32 * b:32 * b + 32, 512 * a:512 * a + 512],
                            start=False, stop=False,
                            tile_position=(32 * b, 32 * a))
                for b in range(4):
                    sl = slice(512 * b, 512 * b + 512)
                    nc.tensor.matmul(
                        out=PZ2[:, sl], lhsT=b2n[32 * b:32 * b + 1,
                                                 128 * sg:128 * sg + 128],
                        rhs=onesH[32 * b:32 * b + 1, :],
                        start=False, stop=True, tile_position=(32 * b, 0))

            def l3_out(sg, h, H2, PC):
                for b in range(4):
                    nc.tensor.matmul(
                        out=PC[32 * b:32 * b + 12, :],
                        lhsT=w3sb[:, 48 * sg + 12 * b:48 * sg + 12 * b + 12],
                        rhs=H2[:, 512 * b:512 * b + 512],
                        start=True, stop=True,
                        tile_position=(0, 32 * b))
                OT = op_.tile([128, NH], f32, tag="OT")
                nc.vector.tensor_scalar(
                    OT[:], PC[:], b3sb[:, sg:sg + 1], None, ALU.add)
                for b in range(4):
                    nc.gpsimd.dma_start(
                        d_out[sg, h, 12 * b:12 * b + 12, :],
                        OT[32 * b:32 * b + 12, :])

            def do_l3(sg, h, H2):
                PC = ps.tile([128, 2048], f32, tag="pz2",
                             name=f"pc_{sg}_{h}")
                l3_out(sg, h, H2, PC[:, 0:NH])

            # Software-pipelined emission. Steady-state ACT stream:
            #   B(0), B(1), D(0), B(2), D(1), ... (no idle gaps)
            # PE stream per iter k: A(k+1), E(k-1), C(k).
            its = [(sg, h) for sg in range(SGS) for h in range(2)]
            nI = len(its)
            PZ1c = ps.tile([128, 2048], f32, tag="pz1", name="pz1_0")
            l1_chain(*its[0], PZ1c)
            H1c = ap.tile([128, 2048], f16, tag="H1", name="H1_0")
            nc.scalar.activation(H1c[:, 0:1024], PZ1c[:, 0:1024], AF.Sin,
                                 bias=0.0, scale=-TWO_PI)
            nc.scalar.activation(H1c[:, 1024:2048], PZ1c[:, 1024:2048],
                                 AF.Sin, bias=0.0, scale=-TWO_PI)
            prevL3 = None
            for k in range(nI):
                sg, h = its[k]
                # C(k) first: unblocks D(k) as early as possible
                PZ2 = ps.tile([128, 2048], f32, tag="pz2",
                              name=f"pz2_{k}")
                l2_chain(sg, h, H1c, PZ2)
                # A(k+1) second: hides under D(k)'s ACT window
                if k + 1 < nI:
                    PZ1n = ps.tile([128, 2048], f32, tag="pz1",
                                   name=f"pz1_{k + 1}")
                    l1_chain(*its[k + 1], PZ1n)
                H2 = ap.tile([128, 2048], f16, tag="H2", name=f"H2_{k}")
                nc.scalar.activation(H2[:], PZ2[:], AF.Sin,
                                     bias=0.0, scale=-TWO_PI)
                if k + 1 < nI:
                    H1n = ap.tile([128, 2048], f16, tag="H1",
                                  name=f"H1_{k + 1}")
                    nc.scalar.activation(H1n[:, 0:1024], PZ1n[:, 0:1024],
                                         AF.Sin, bias=0.0, scale=-TWO_PI)
                    nc.scalar.activation(H1n[:, 1024:2048],
                                         PZ1n[:, 1024:2048],
                                         AF.Sin, bias=0.0, scale=-TWO_PI)
                # L3 of the previous iter LAST: its PC allocation then
                # queues behind pz2(k) in the tag rotation, so C(k+1)
                # never waits on the DVE copy of PC(k-1).
                if prevL3 is not None:
                    do_l3(*prevL3)
                prevL3 = (sg, h, H2)
                if k + 1 < nI:
                    PZ1c, H1c = PZ1n, H1n
            do_l3(*prevL3)

    nc.compile()
    return nc


def _split_bf16(x):
    """x (fp32) -> (hi, lo) bf16 pair with hi+lo ~= x."""
    hi = x.astype(ml_dtypes.bfloat16)
    lo = (x - hi.astype(np.float32)).astype(ml_dtypes.bfloat16)
    return hi, lo


def _prep_core_inputs(w1, b1, w2, b2, w3, b3, coords, core):
    s = np.float64(W0 / TWO_PI)
    B0 = core * BPC
    sl = slice(B0, B0 + BPC)

    # net (sg, a, b) = batch B0 + 16sg + 4a + b
    # L1: lhsT strip a rows 32a+k, cols 128sg + 32b + u
    w1c = (w1[sl, :, :, 0].astype(np.float64) * s).astype(np.float32)
    w1c = w1c.reshape(SGS, 4, 4, IN, H)          # [sg,a,b,i,u]
    b1c = (b1[sl, :, 0].astype(np.float64) * s).astype(np.float32)
    b1c = b1c.reshape(SGS, 4, 4, H)              # [sg,a,b,u]
    w1h, w1l = _split_bf16(w1c.astype(np.float32))
    b1h, b1l = _split_bf16(b1c.astype(np.float32))
    # rows: [w1h(x1h), w1h(x1l), w1l(x1h), w2h(x2h), w2h(x2l), w2l(x2h), bh, bl]
    aug = np.zeros((SGS, 4, 4, 9, H), ml_dtypes.bfloat16)
    aug[:, :, :, 0] = w1h[:, :, :, 0]
    aug[:, :, :, 1] = w1h[:, :, :, 0]
    aug[:, :, :, 2] = w1l[:, :, :, 0]
    aug[:, :, :, 3] = w1h[:, :, :, 1]
    aug[:, :, :, 4] = w1h[:, :, :, 1]
    aug[:, :, :, 5] = w1l[:, :, :, 1]
    aug[:, :, :, 6] = b1h
    aug[:, :, :, 7] = b1l
    aug[:, :, :, 8] = MAGIC
    w1aug = np.ascontiguousarray(
        aug.transpose(1, 3, 0, 2, 4).reshape(4, 9, SGS * 128))

    # L2 weights: strip b rows 32b+i, cols 128sg + 32a + o -> net (a,b)
    w2c = (w2[sl, :, :, 0].astype(np.float64) * s).astype(np.float16)
    w2c = w2c.reshape(SGS, 4, 4, H, H)           # [sg,a,b,i,o]
    w2s = np.ascontiguousarray(
        w2c.transpose(2, 3, 0, 1, 4).reshape(4, 32, SGS * 128))

    # L2 bias, pre-reduced mod 1 (exact w.r.t. sin): row per strip b,
    # col 128sg + 32a + o
    b2c = (b2[sl, :, 0].astype(np.float64) * s)
    b2c = b2c - np.round(b2c)                     # [-0.5, 0.5]
    b2c = b2c.astype(np.float16).reshape(SGS, 4, 4, H)   # [sg,a,b,o]
    b2prep = np.ascontiguousarray(
        b2c.transpose(2, 0, 1, 3).reshape(4, SGS * 128))

    # L3 block-diag per (sg, b): [128, 12]: [32a+i, 3a'+c] = w3[net(sg,a,b)]
    w3c = w3[sl, :, :, 0].astype(np.float16).reshape(SGS, 4, 4, H, OUT)
    blk = np.zeros((SGS, 4, 4, H, 4, OUT), np.float16)  # [sg,a,b,i,a',c]
    for a in range(4):
        blk[:, a, :, :, a, :] = w3c[:, a]
    # free inside sg block: 12*b + 3*a' + c ; partition 32*a + i
    w3blk = np.ascontiguousarray(
        blk.transpose(1, 3, 0, 2, 4, 5).reshape(4, 32, SGS * 48))

    # L3 bias: PC partition 32b + 3a + c
    b3c = b3[sl, :, 0].astype(np.float32).reshape(SGS, 4, 4, OUT)  # [sg,a,b,c]
    b3prep = np.zeros((128, SGS), np.float32)
    p = np.arange(128)
    b_idx, m_idx = p // 32, p % 32
    a3, c3 = m_idx // 3, m_idx % 3
    for pi in range(128):
        if m_idx[pi] < 12:
            b3prep[pi, :] = b3c[:, a3[pi], b_idx[pi], c3[pi]]

    # coords rows per strip: [x1h, x1l, x1h, x2h, x2l, x2h, 1, 1]
    x = coords.astype(np.float32)                # [N, 2]
    xh, xl = _split_bf16(x)
    ch = np.zeros((4, 9, N), ml_dtypes.bfloat16)
    for a in range(4):
        ch[a, 0] = xh[:, 0]
        ch[a, 1] = xl[:, 0]
        ch[a, 2] = xh[:, 0]
        ch[a, 3] = xh[:, 1]
        ch[a, 4] = xl[:, 1]
        ch[a, 5] = xh[:, 1]
        ch[a, 6] = 1.0
        ch[a, 7] = 1.0
        ch[a, 8] = 1.0

    return {"w1aug": w1aug, "w2s": w2s, "b2p": b2prep, "w3blk": w3blk,
            "b3c": b3prep, "coords": ch}


def _unshard(res_list):
    outs = []
    for r in res_list:
        o = r["out"].reshape(SGS, 2, 4, 4, OUT, NH)      # [sg,h,b,a,c,n]
        o = o.transpose(0, 3, 2, 1, 5, 4)                # [sg,a,b,h,n,c]
        outs.append(np.ascontiguousarray(
            o.reshape(BPC, N, OUT).astype(np.float32)))
    return np.concatenate(outs, axis=0)


def _run(inputs, trace=False, trace_kwargs=None):
    global _compiled
    if _compiled is None:
        _compiled = _build_module()
    nc = _compiled
    arrs = {k: np.asarray(v, dtype=np.float32) for k, v in inputs.items()}
    in_maps = [_prep_core_inputs(arrs["w1"], arrs["b1"], arrs["w2"],
                                 arrs["b2"], arrs["w3"], arrs["b3"],
                                 arrs["coords"], c)
               for c in range(N_CORES)]
    kw = {}
    if trace:
        kw["trace"] = True
        if trace_kwargs:
            kw.update(trace_kwargs)
    res = bass_utils.run_bass_kernel_spmd(nc, in_maps,
                                          core_ids=list(range(N_CORES)), **kw)
    out = _unshard(res.results)
    return out, res


def kernel(**inputs):
    out, _ = _run(inputs, trace=False)
    return out


# revision 18
# speedup vs baseline: 1.0035x; 1.0035x over previous
"""BatchSiren Trainium2 kernel, v2.

B=2048 independent SIREN MLPs (2->32->32->3, sin activations, w0=30),
each evaluated on the same N=1024 coordinate grid. Pure data parallel
over 8 cores, 256 nets/core; 16 supergroups (sg) of 16 nets; two
point-halves (h) of 512.

Core idea: range reduction for sin is done INSIDE the PE array via a
"magic sandwich" of chained accumulating matmuls into the same PSUM
bank:  [+z] [+MAGIC] [-MAGIC] [-z]  leaves  round(z) - z = -r  in PSUM
(each matmul's contribution is one fp32 add into PSUM; the +MAGIC add
rounds the accumulator to an integer, bf16 holds MAGIC exactly).
ScalarE then does a single Sin pass per element (the only per-element
engine pass in the whole kernel): sin(-2pi * (-r)) = sin(2pi z).

L1 uses a single K=8 bf16 matmul with hi/lo-split weights AND coords
(w = wh + wl, x = xh + xl, dropping the lo*lo term) which reproduces
the fp32 product to ~2^-18 relative - needed because L1 errors amplify
~17x through layer 2. L2/L3 run in fp16 (errors don't amplify).

Per (sg,h) iteration:
  L1: 4-way row-group packs: [z K=8 bf16][+M K=1][-M K=1][-z K=8]
      -> PZ1 [128,2048] (4 banks); ACT Sin FD=2048 -> H1 f16
  L2: per bank b: [+b2 K=1 f16][z K=32 f16 16-tile][+M][-M][-z][-b2]
      -> PZ2 [128,2048]; ACT Sin FD=2048 -> H2 f16
  L3: 4 col-tiled K=128 f16 matmuls -> PC [128,512]; DVE ts adds b3
      (per-partition scalar) and copies to SBUF; DMA out.
"""
import numpy as np
import ml_dtypes

import concourse.bacc as bacc
import concourse.bass as bass
import concourse.mybir as mybir
import concourse.tile as tile
from concourse import bass_utils
from bass_rust import add_dep_helper

f32 = mybir.dt.float32
f16 = mybir.dt.float16
bf16 = mybir.dt.bfloat16
AF = mybir.ActivationFunctionType
ALU = mybir.AluOpType

W0 = 30.0
MAGIC = float(1.5 * 2 ** 23)
TWO_PI = float(2.0 * np.pi)
N_CORES = 8
B, N, IN, H, OUT = 2048, 1024, 2, 32, 3
BPC = B // N_CORES        # 256 nets per core
SGS = BPC // 16           # 16 supergroups of 16 nets
NH = N // 2               # 512 points per half

_compiled = None


def _build_module():
    nc = bacc.Bacc("TRN2", target_bir_lowering=False, debug=False)

    d_w1 = nc.dram_tensor("w1aug", [4, 9, 128 * SGS], bf16, kind="ExternalInput")
    d_w2 = nc.dram_tensor("w2s", [4, 32, 128 * SGS], f16, kind="ExternalInput")
    d_b2 = nc.dram_tensor("b2p", [4, 128 * SGS], f16, kind="ExternalInput")
    d_w3 = nc.dram_tensor("w3blk", [4, 32, 48 * SGS], f16, kind="ExternalInput")
    d_b3 = nc.dram_tensor("b3c", [128, SGS], f32, kind="ExternalInput")
    d_c = nc.dram_tensor("coords", [4, 9, N], bf16, kind="ExternalInput")
    d_out = nc.dram_tensor("out", [SGS, 2, 48, NH], f32, kind="ExternalOutput")

    with tile.TileContext(nc, pool_alloc_mode="queue") as tc:
        with tc.tile_pool(name="const", bufs=1) as cp, \
             tc.tile_pool(name="acts", bufs=3) as ap, \
             tc.tile_pool(name="outp", bufs=8) as op_, \
             tc.tile_pool(name="ps", bufs=1, space="PSUM") as ps:

            # ---- persistent constants ----
            # L1-critical pieces first so iteration 0 starts ASAP:
            c4x = cp.tile([128, N], bf16, tag="c4x")
            nc.vector.memset(c4x[:], 0.0)
            for a in range(4):
                nc.sync.dma_start(c4x[32 * a:32 * a + 9, :], d_c[a])
            mgP = cp.tile([128, 128], bf16, tag="mgP")
            nc.vector.memset(mgP[:], MAGIC)
            mgN = cp.tile([128, 128], bf16, tag="mgN")
            nc.vector.memset(mgN[:], -MAGIC)
            onesB = cp.tile([128, NH], bf16, tag="onesB")
            nc.vector.memset(onesB[:], 1.0)
            onesH = cp.tile([128, NH], f16, tag="onesH")
            nc.vector.memset(onesH[:], 1.0)

            w1p = cp.tile([128, 128 * SGS], bf16, tag="w1p")
            for a in range(4):
                nc.sync.dma_start(w1p[32 * a:32 * a + 9, :], d_w1[a])
            w1n = cp.tile([128, 128 * SGS], bf16, tag="w1n")
            nc.vector.tensor_scalar(w1n[:], w1p[:], -1.0, None, ALU.mult)

            b2p = cp.tile([128, 128 * SGS], f16, tag="b2p")
            for b in range(4):
                nc.sync.dma_start(b2p[32 * b:32 * b + 1, :], d_b2[b])
            w2p = cp.tile([128, 128 * SGS], f16, tag="w2p")
            for b in range(4):
                nc.sync.dma_start(w2p[32 * b:32 * b + 32, :], d_w2[b])
            b2n = cp.tile([128, 128 * SGS], f16, tag="b2n")
            nc.vector.tensor_scalar(b2n[:], b2p[:], -1.0, None, ALU.mult)
            w2n = cp.tile([128, 128 * SGS], f16, tag="w2n")
            nc.vector.tensor_scalar(w2n[:], w2p[:], -1.0, None, ALU.mult)

            w3sb = cp.tile([128, 48 * SGS], f16, tag="w3")
            for a in range(4):
                nc.sync.dma_start(w3sb[32 * a:32 * a + 32, :], d_w3[a])
            b3sb = cp.tile([128, SGS], f32, tag="b3")
            nc.sync.dma_start(b3sb[:], d_b3[:])

            def l1_chain(sg, h, PZ1):
                last = None
                # per bank a: [z;+M K=9] [-M K=1] [-z K=8]; the +M row
                # rounds INSIDE the K=9 matmul (internal row-order accum).
                for a in range(4):
                    sl = slice(512 * a, 512 * a + 512)
                    nc.tensor.matmul(
                        out=PZ1[:, sl],
                        lhsT=w1p[32 * a:32 * a + 9,
                                 128 * sg:128 * sg + 128],
                        rhs=c4x[32 * a:32 * a + 9, NH * h:NH * h + NH],
                        start=True, stop=False, tile_position=(32 * a, 0))
                for a in range(4):
                    sl = slice(512 * a, 512 * a + 512)
                    nc.tensor.matmul(
                        out=PZ1[:, sl], lhsT=mgN[32 * a:32 * a + 1, :],
                        rhs=onesB[32 * a:32 * a + 1, :],
                        start=False, stop=False, tile_position=(32 * a, 0))
                for a in range(4):
                    sl = slice(512 * a, 512 * a + 512)
                    last = nc.tensor.matmul(
                        out=PZ1[:, sl],
                        lhsT=w1n[32 * a:32 * a + 8,
                                 128 * sg:128 * sg + 128],
                        rhs=c4x[32 * a:32 * a + 8, NH * h:NH * h + NH],
                        start=False, stop=True, tile_position=(32 * a, 0))
                return last

            def l2_chain(sg, h, H1, PZ2, after=None):
                first = None
                for b in range(4):
                    sl = slice(512 * b, 512 * b + 512)
                    mm = nc.tensor.matmul(
                        out=PZ2[:, sl], lhsT=b2p[32 * b:32 * b + 1,
                                                 128 * sg:128 * sg + 128],
                        rhs=onesH[32 * b:32 * b + 1, :],
                        start=True, stop=False, tile_position=(32 * b, 0))
                    if first is None:
                        first = mm
                # z: 16 tiles (b strip, a colgroup); a-outer so tiles
                # reading H1's first half can start before the second
                # Sin-half completes (subtile deps).
                for a in range(4):
                    for b in range(4):
                        nc.tensor.matmul(
                            out=PZ2[32 * a:32 * a + 32, 512 * b:512 * b + 512],
                            lhsT=w2p[32 * b:32 * b + 32,
                                     128 * sg + 32 * a:128 * sg + 32 * a + 32],
                            rhs=H1[32 * b:32 * b + 32, 512 * a:512 * a + 512],
                            start=False, stop=False,
                            tile_position=(32 * b, 32 * a))
                for b in range(4):
                    sl = slice(512 * b, 512 * b + 512)
                    nc.tensor.matmul(
                        out=PZ2[:, sl], lhsT=mgP[32 * b:32 * b + 1, :],
                        rhs=onesB[32 * b:32 * b + 1, :],
                        start=False, stop=False, tile_position=(32 * b, 0))
                for b in range(4):
                    sl = slice(512 * b, 512 * b + 512)
                    nc.tensor.matmul(
                        out=PZ2[:, sl], lhsT=mgN[32 * b:32 * b + 1, :],
                        rhs=onesB[32 * b:32 * b + 1, :],
                        start=False, stop=False, tile_position=(32 * b, 0))
                for b in range(4):
                    for a in range(4):
                        nc.tensor.matmul(
                            out=PZ2[32 * a:32 * a + 32, 512 * b:512 * b + 512],
                            lhsT=w2n[32 * b:32 * b + 32,
                                     128 * sg + 32 * a:128 * sg + 32 * a + 32],
                            rhs=H1[32 * b:32 * b + 32, 512 * a:512 * a + 512],
                            start=False, stop=False,
                            tile_position=(32 * b, 32 * a))
                for b in range(4):
                    sl = slice(512 * b, 512 * b + 512)
                    nc.tensor.matmul(
                        out=PZ2[:, sl], lhsT=b2n[32 * b:32 * b + 1,
                                                 128 * sg:128 * sg + 128],
                        rhs=onesH[32 * b:32 * b + 1, :],
                        start=False, stop=True, tile_position=(32 * b, 0))

            def l3_out(sg, h, H2, PC):
                for b in range(4):
                    nc.tensor.matmul(
                        out=PC[32 * b:32 * b + 12, :],
                        lhsT=w3sb[:, 48 * sg + 12 * b:48 * sg + 12 * b + 12],
                        rhs=H2[:, 512 * b:512 * b + 512],
                        start=True, stop=True,
                        tile_position=(0, 32 * b))
                OT = op_.tile([128, NH], f32, tag="OT")
                nc.vector.tensor_scalar(
                    OT[:], PC[:], b3sb[:, sg:sg + 1], None, ALU.add)
                for b in range(4):
                    nc.gpsimd.dma_start(
                        d_out[sg, h, 12 * b:12 * b + 12, :],
                        OT[32 * b:32 * b + 12, :])

            def do_l3(sg, h, H2):
                PC = ps.tile([128, 2048], f32, tag="pz2",
                             name=f"pc_{sg}_{h}")
                l3_out(sg, h, H2, PC[:, 0:NH])

            # Software-pipelined emission. Steady-state ACT stream:
            #   B(0), B(1), D(0), B(2), D(1), ... (no idle gaps)
            # PE stream per iter k: A(k+1), E(k-1), C(k).
            its = [(sg, h) for sg in range(SGS) for h in range(2)]
            nI = len(its)
            PZ1c = ps.tile([128, 2048], f32, tag="pz1", name="pz1_0")
            l1_chain(*its[0], PZ1c)
            H1c = ap.tile([128, 2048], f16, tag="H1", name="H1_0")
            nc.scalar.activation(H1c[:, 0:1024], PZ1c[:, 0:1024], AF.Sin,
                                 bias=0.0, scale=-TWO_PI)
            nc.scalar.activation(H1c[:, 1024:2048], PZ1c[:, 1024:2048],
                                 AF.Sin, bias=0.0, scale=-TWO_PI)
            prevL3 = None
            for k in range(nI):
                sg, h = its[k]
                if prevL3 is not None:
                    do_l3(*prevL3)
                    prevL3 = None
                # C(k) first: unblocks D(k) as early as possible
                PZ2 = ps.tile([128, 2048], f32, tag="pz2",
                              name=f"pz2_{k}")
                l2_chain(sg, h, H1c, PZ2)
                # A(k+1) second: hides under D(k)'s ACT window
                if k + 1 < nI:
                    PZ1n = ps.tile([128, 2048], f32, tag="pz1",
                                   name=f"pz1_{k + 1}")
                    l1_chain(*its[k + 1], PZ1n)
                H2 = ap.tile([128, 2048], f16, tag="H2", name=f"H2_{k}")
                nc.scalar.activation(H2[:], PZ2[:], AF.Sin,
                                     bias=0.0, scale=-TWO_PI)
                if k + 1 < nI:
                    H1n = ap.tile([128, 2048], f16, tag="H1",
                                  name=f"H1_{k + 1}")
                    nc.scalar.activation(H1n[:, 0:1024], PZ1n[:, 0:1024],
                                         AF.Sin, bias=0.0, scale=-TWO_PI)
                    nc.scalar.activation(H1n[:, 1024:2048],
                                         PZ1n[:, 1024:2048],
                                         AF.Sin, bias=0.0, scale=-TWO_PI)
                prevL3 = (sg, h, H2)
                if k + 1 < nI:
                    PZ1c, H1c = PZ1n, H1n
            do_l3(*prevL3)

    nc.compile()
    return nc


def _split_bf16(x):
    """x (fp32) -> (hi, lo) bf16 pair with hi+lo ~= x."""
    hi = x.astype(ml_dtypes.bfloat16)
    lo = (x - hi.astype(np.float32)).astype(ml_dtypes.bfloat16)
    return hi, lo


def _prep_core_inputs(w1, b1, w2, b2, w3, b3, coords, core):
    s = np.float64(W0 / TWO_PI)
    B0 = core * BPC
    sl = slice(B0, B0 + BPC)

    # net (sg, a, b) = batch B0 + 16sg + 4a + b
    # L1: lhsT strip a rows 32a+k, cols 128sg + 32b + u
    w1c = (w1[sl, :, :, 0].astype(np.float64) * s).astype(np.float32)
    w1c = w1c.reshape(SGS, 4, 4, IN, H)          # [sg,a,b,i,u]
    b1c = (b1[sl, :, 0].astype(np.float64) * s).astype(np.float32)
    b1c = b1c.reshape(SGS, 4, 4, H)              # [sg,a,b,u]
    w1h, w1l = _split_bf16(w1c.astype(np.float32))
    b1h, b1l = _split_bf16(b1c.astype(np.float32))
    # rows: [w1h(x1h), w1h(x1l), w1l(x1h), w2h(x2h), w2h(x2l), w2l(x2h), bh, bl]
    aug = np.zeros((SGS, 4, 4, 9, H), ml_dtypes.bfloat16)
    aug[:, :, :, 0] = w1h[:, :, :, 0]
    aug[:, :, :, 1] = w1h[:, :, :, 0]
    aug[:, :, :, 2] = w1l[:, :, :, 0]
    aug[:, :, :, 3] = w1h[:, :, :, 1]
    aug[:, :, :, 4] = w1h[:, :, :, 1]
    aug[:, :, :, 5] = w1l[:, :, :, 1]
    aug[:, :, :, 6] = b1h
    aug[:, :, :, 7] = b1l
    aug[:, :, :, 8] = MAGIC
    w1aug = np.ascontiguousarray(
        aug.transpose(1, 3, 0, 2, 4).reshape(4, 9, SGS * 128))

    # L2 weights: strip b rows 32b+i, cols 128sg + 32a + o -> net (a,b)
    w2c = (w2[sl, :, :, 0].astype(np.float64) * s).astype(np.float16)
    w2c = w2c.reshape(SGS, 4, 4, H, H)           # [sg,a,b,i,o]
    w2s = np.ascontiguousarray(
        w2c.transpose(2, 3, 0, 1, 4).reshape(4, 32, SGS * 128))

    # L2 bias, pre-reduced mod 1 (exact w.r.t. sin): row per strip b,
    # col 128sg + 32a + o
    b2c = (b2[sl, :, 0].astype(np.float64) * s)
    b2c = b2c - np.round(b2c)                     # [-0.5, 0.5]
    b2c = b2c.astype(np.float16).reshape(SGS, 4, 4, H)   # [sg,a,b,o]
    b2prep = np.ascontiguousarray(
        b2c.transpose(2, 0, 1, 3).reshape(4, SGS * 128))

    # L3 block-diag per (sg, b): [128, 12]: [32a+i, 3a'+c] = w3[net(sg,a,b)]
    w3c = w3[sl, :, :, 0].astype(np.float16).reshape(SGS, 4, 4, H, OUT)
    blk = np.zeros((SGS, 4, 4, H, 4, OUT), np.float16)  # [sg,a,b,i,a',c]
    for a in range(4):
        blk[:, a, :, :, a, :] = w3c[:, a]
    # free inside sg block: 12*b + 3*a' + c ; partition 32*a + i
    w3blk = np.ascontiguousarray(
        blk.transpose(1, 3, 0, 2, 4, 5).reshape(4, 32, SGS * 48))

    # L3 bias: PC partition 32b + 3a + c
    b3c = b3[sl, :, 0].astype(np.float32).reshape(SGS, 4, 4, OUT)  # [sg,a,b,c]
    b3prep = np.zeros((128, SGS), np.float32)
    p = np.arange(128)
    b_idx, m_idx = p // 32, p % 32
    a3, c3 = m_idx // 3, m_idx % 3
    for pi in range(128):
        if m_idx[pi] < 12:
            b3prep[pi, :] = b3c[:, a3[pi], b_idx[pi], c3[pi]]

    # coords rows per strip: [x1h, x1l, x1h, x2h, x2l, x2h, 1, 1]
    x = coords.astype(np.float32)                # [N, 2]
    xh, xl = _split_bf16(x)
    ch = np.zeros((4, 9, N), ml_dtypes.bfloat16)
    for a in range(4):
        ch[a, 0] = xh[:, 0]
        ch[a, 1] = xl[:, 0]
        ch[a, 2] = xh[:, 0]
        ch[a, 3] = xh[:, 1]
        ch[a, 4] = xl[:, 1]
        ch[a, 5] = xh[:, 1]
        ch[a, 6] = 1.0
        ch[a, 7] = 1.0
        ch[a, 8] = 1.0

    return {"w1aug": w1aug, "w2s": w2s, "b2p": b2prep, "w3blk": w3blk,
            "b3c": b3prep, "coords": ch}


def _unshard(res_list):
    outs = []
    for r in res_list:
        o = r["out"].reshape(SGS, 2, 4, 4, OUT, NH)      # [sg,h,b,a,c,n]
        o = o.transpose(0, 3, 2, 1, 5, 4)                # [sg,a,b,h,n,c]
        outs.append(np.ascontiguousarray(
            o.reshape(BPC, N, OUT).astype(np.float32)))
    return np.concatenate(outs, axis=0)


def _run(inputs, trace=False, trace_kwargs=None):
    global _compiled
    if _compiled is None:
        _compiled = _build_module()
    nc = _compiled
    arrs = {k: np.asarray(v, dtype=np.float32) for k, v in inputs.items()}
    in_maps = [_prep_core_inputs(arrs["w1"], arrs["b1"], arrs["w2"],
                                 arrs["b2"], arrs["w3"], arrs["b3"],
                                 arrs["coords"], c)
               for c in range(N_CORES)]
    kw = {}
    if trace:
        kw["trace"] = True
        if trace_kwargs:
            kw.update(trace_kwargs)
    res = bass_utils.run_bass_kernel_spmd(nc, in_maps,
                                          core_ids=list(range(N_CORES)), **kw)
    out = _unshard(res.results)
    return out, res


def kernel(**inputs):
    out, _ = _run(inputs, trace=False)
    return out
